# revision 3
# baseline (speedup 1.0000x reference)
"""Trainium2 Bass kernel for masked attention softmax (ragged sequences).

Reference computation (per batch b):
    qp[k]   = sum_q query[b,0,q] * w[k,q]
    att[s]  = sum_k qp[k] * keys[b,s,k]
    score   = where(s < seq_len[b], att, NEG_INF)
    out[b]  = softmax(score)            # over s axis

Strategy (v2 -- fp16 tensor_tensor + tree-fold):
  - Data-parallel over batch across 8 cores (512 batches/core, 4 slot tiles
    of 128 on the partition dim).  Batches sorted by seq_len descending and
    dealt round-robin so slot j has the same extent E_j on every core; keys
    zero-padded to E_j.
  - All big tensors fp16: halves both DMA (16.4MB/core) and the DVE
    multiply pass.  scalar_tensor_tensor (old kernel's core op) has NO DVE
    perf mode (1 elem/cycle), but plain tensor_tensor runs 2x_1P with
    all-fp16 operands, so the dot product is restructured as
      prod = kt * qp_bcast                  (TT mult fp16 2x, 0-stride qp)
      h1..h4: fold 128 -> 64 -> 32 -> 16 -> 8  (TT add fp16 2x)
      att    = tensor_reduce(h4, axis=X)    (1x over 8-wide, fp32 out --
               the fp32-internal accumulator finishes the sum exactly)
    ~132 DVE cyc/position vs 188+ for STT, in 6 instructions per 64-position
    chunk instead of one per position.
  - Mask: att += maskpen (0 valid / -1e9 padded) before exp, so padded
    positions contribute exp(-1e9)=0 to the softmax sum exactly.  (A
    pad-count subtraction is catastrophically cancelled in fp32 when the
    valid exp-sum is tiny -- e.g. seq_len==1 rows with negative logits.)
  - qp on device (PE): per slot one fp16 matmul [qT | wT] -> PSUM fp32,
    cast to fp16 by ACT.
  - softmax: ACT exp with accum_out, DVE reciprocal, ACT scale; fp16 output
    via SWDGE so the HWDGE queues carry only keys chunks.
  - keys chunks alternate between the two HWDGE rings (sync + scalar) so a
    buffer-recycle wait on one queue head doesn't stall the other.
  - Host: sort/deal, fp16 convert + zero-pad, unpack, zero the masked tail,
    seq_len==0 rows are uniform 1/S.
"""

import sys

import numpy as np

sys.path.insert(0, "/opt/trn_rl_repo")

import concourse.bass as bass
import concourse.tile as tile
from concourse import bacc, mybir
from concourse.bass_utils import run_bass_kernel_spmd


def _install_trace_shims():
    """The agent image lacks ``antenv.axon_hooks``, so trace=True silently
    degrades.  Recreate the module and register the ctypes NTFF hook from
    trn_agent_boot; also make artifact upload failure non-fatal."""
    try:
        import types

        import antenv
        from concourse import bass_utils as _bu

        if "antenv.axon_hooks" not in sys.modules:
            mod = types.ModuleType("antenv.axon_hooks")
            mod._hook = None
            mod.set_axon_ntff_profile_hook = lambda h: setattr(mod, "_hook", h)
            mod.get_axon_ntff_profile_hook = lambda: mod._hook
            sys.modules["antenv.axon_hooks"] = mod
            antenv.axon_hooks = mod
            from trn_agent_boot.trn_boot import _ntff_profile_via_ctypes

            mod.set_axon_ntff_profile_hook(
                _ntff_profile_via_ctypes("/opt/axon/libaxon_pjrt.so")
            )

        _orig_upload = _bu.upload_artifacts

        def _safe_upload(tmpdir):
            try:
                return _orig_upload(tmpdir)
            except Exception:
                return "local://" + str(tmpdir)

        _bu.upload_artifacts = _safe_upload
    except Exception:
        pass


_install_trace_shims()

B, S, KD, QD = 4096, 200, 128, 128
NCORES = 8
P = 128
PB = B // NCORES           # batches per core
NTILES = PB // P           # slot tiles per core
CH = 64                    # s-positions per keys DMA / DVE chunk
MASK_NEG = -1.0e9

LAST_RESULTS = None
_nc_cache = {}


def _chunks_for(E, first_slot):
    """Chunk plan along the position axis; geometric ramp on the first slot
    so the DVE starts as soon as the first ~0.25MB of keys has landed."""
    plan = []
    c0 = 0
    if first_slot:
        for ch in (8, 16, 32):
            if c0 + ch <= E:
                plan.append((c0, ch))
                c0 += ch
    while c0 < E:
        ch = min(CH, E - c0)
        plan.append((c0, ch))
        c0 += ch
    return plan


def _build(s_exts):
    f16 = mybir.dt.float16
    f32 = mybir.dt.float32
    SE = sum(s_exts)
    nc = bacc.Bacc("TRN2", target_bir_lowering=False, debug=False)

    keys_d = nc.dram_tensor("keys", [P, SE, KD], f16, kind="ExternalInput")
    # qw[j] = [qT_j | wT] fused so each slot's matmul depends on ONE dma.
    qw_d = nc.dram_tensor("qw", [QD, NTILES, P + KD], f16, kind="ExternalInput")
    mp_d = nc.dram_tensor("mp", [P, SE], f32, kind="ExternalInput")
    out_d = nc.dram_tensor("out", [P, SE], f16, kind="ExternalOutput")

    with nc.allow_low_precision(reason="fp16 tree-fold; tensor_reduce tail is fp32"):
        with tile.TileContext(nc) as tc:
            with (
                tc.tile_pool(name="keys", bufs=4) as keysp,
                tc.tile_pool(name="prod", bufs=2) as prodp,
                tc.tile_pool(name="h16", bufs=2) as h16p,
                tc.tile_pool(name="h32", bufs=2) as h32p,
                tc.tile_pool(name="small", bufs=2) as smallp,
                tc.tile_pool(name="soft", bufs=2) as softp,
                tc.tile_pool(name="qpp", bufs=NTILES) as qpp,
                tc.tile_pool(name="psum", bufs=2, space=bass.MemorySpace.PSUM) as psump,
            ):
                qw = smallp.tile([QD, NTILES, P + KD], f16, tag="qw")
                nc.sync.dma_start(qw[:], qw_d[:])
                mp_t = smallp.tile([P, SE], f32, tag="mp")
                nc.scalar.dma_start(mp_t[:], mp_d[:])

                # qp for ALL slots up-front (PE otherwise idle).
                qps = []
                for j in range(NTILES):
                    qp_ps = psump.tile([P, KD], f32, tag="qp_ps")
                    nc.tensor.matmul(
                        qp_ps[:], qw[:, j, :P], qw[:, j, P : P + KD],
                        start=True, stop=True,
                    )
                    qp = qpp.tile([P, KD], f16, tag=f"qp{j}")
                    nc.scalar.copy(qp[:], qp_ps[:])
                    qps.append(qp)

                off = 0
                qidx = 0
                for j in range(NTILES):
                    E = s_exts[j]
                    qp = qps[j]
                    att = softp.tile([P, E], f32, tag="att")

                    for c0, ch in _chunks_for(E, j == 0):
                        kt = keysp.tile([P, CH, KD], f16, tag="kt")
                        dma_eng = nc.sync if (qidx % 2 == 0) else nc.scalar
                        qidx += 1
                        dma_eng.dma_start(
                            kt[:, :ch, :], keys_d[:, off + c0 : off + c0 + ch, :]
                        )
                        # prod = kt * qp  (qp broadcast along the position dim)
                        prod = prodp.tile([P, CH, KD], f16, tag="prod")
                        nc.vector.tensor_tensor(
                            prod[:, :ch, :],
                            kt[:, :ch, :],
                            qp[:].unsqueeze(1).broadcast_to([P, ch, KD]),
                            op=mybir.AluOpType.mult,
                        )
                        # fp16 2x folds: 128 -> 64 -> 32 -> 16 -> 8
                        h1 = h16p.tile([P, CH, 64], f16, tag="h1")
                        nc.vector.tensor_tensor(
                            h1[:, :ch, :], prod[:, :ch, 0:64], prod[:, :ch, 64:128],
                            op=mybir.AluOpType.add,
                        )
                        h2 = h32p.tile([P, CH, 32], f16, tag="h2")
                        nc.vector.tensor_tensor(
                            h2[:, :ch, :], h1[:, :ch, 0:32], h1[:, :ch, 32:64],
                            op=mybir.AluOpType.add,
                        )
                        h3 = h32p.tile([P, CH, 16], f16, tag="h3")
                        nc.vector.tensor_tensor(
                            h3[:, :ch, :], h2[:, :ch, 0:16], h2[:, :ch, 16:32],
                            op=mybir.AluOpType.add,
                        )
                        h4 = h16p.tile([P, CH, 8], f16, tag="h4")
                        nc.vector.tensor_tensor(
                            h4[:, :ch, :], h3[:, :ch, 0:8], h3[:, :ch, 8:16],
                            op=mybir.AluOpType.add,
                        )
                        # finish the 8-wide sums in the fp32 accumulator
                        nc.vector.tensor_reduce(
                            att[:, c0 : c0 + ch], h4[:, :ch, :],
                            axis=mybir.AxisListType.X, op=mybir.AluOpType.add,
                        )

                    # mask padded positions to -1e9 so they exp to 0 exactly
                    atm = softp.tile([P, E], f32, tag="atm")
                    nc.vector.tensor_tensor(
                        atm[:], att[:], mp_t[:, off : off + E],
                        op=mybir.AluOpType.add,
                    )
                    # softmax without max-subtraction: |att| <= ~60 so exp is
                    # finite in fp32 and softmax is shift-invariant.
                    e_t = softp.tile([P, E], f32, tag="e")
                    ssum = softp.tile([P, 1], f32, tag="ssum")
                    nc.scalar.activation(
                        e_t[:], atm[:], mybir.ActivationFunctionType.Exp,
                        bias=0.0, scale=1.0, accum_out=ssum[:],
                    )
                    rec = softp.tile([P, 1], f32, tag="rec")
                    nc.vector.reciprocal(rec[:], ssum[:])
                    o_t = softp.tile([P, E], f16, tag="o")
                    nc.scalar.mul(o_t[:], e_t[:], rec[:])
                    nc.gpsimd.dma_start(out_d[:, off : off + E], o_t[:])
                    off += E
    nc.compile()
    return nc


def _prep(query, keys, seq_len, w):
    query = np.asarray(query)
    keys = np.asarray(keys)
    w = np.asarray(w)
    lens = np.asarray(seq_len).reshape(B).astype(np.int64)

    order = np.argsort(-lens, kind="stable")
    gp = NCORES * P  # batches per slot across all cores
    slot_max = [int(lens[order[j * gp : (j + 1) * gp]].max()) for j in range(NTILES)]
    s_exts = tuple(min(S, max(1, m)) for m in slot_max)
    SE = sum(s_exts)

    perms = []
    for c in range(NCORES):
        perms.append(
            np.concatenate(
                [order[j * gp : (j + 1) * gp][c::NCORES] for j in range(NTILES)]
            )
        )

    keys16 = keys.astype(np.float16)
    q16 = query[:, 0, :].astype(np.float16)
    wT16 = np.ascontiguousarray(w.T).astype(np.float16)
    arange_s = np.arange(S)[None, :]

    in_maps = []
    for c in range(NCORES):
        pc = perms[c]
        ka = np.zeros((P, SE, KD), dtype=np.float16)
        mp = np.zeros((P, SE), dtype=np.float32)
        qw = np.empty((QD, NTILES, P + KD), dtype=np.float16)
        off = 0
        for j in range(NTILES):
            E = s_exts[j]
            rows = pc[j * P : (j + 1) * P]
            sl = np.minimum(lens[rows], E)
            blk = keys16[rows, :E, :]
            blk = np.where((arange_s[:, :E, None] < sl[:, None, None]), blk, 0)
            ka[:, off : off + E, :] = blk
            mp[:, off : off + E] = np.where(
                arange_s[:, :E] < sl[:, None], 0.0, np.float32(MASK_NEG)
            )
            qw[:, j, :P] = q16[rows].T
            qw[:, j, P:] = wT16
            off += E
        in_maps.append({"keys": ka, "qw": qw, "mp": mp})
    return lens, s_exts, perms, in_maps


def kernel(query, keys, seq_len, w):
    global LAST_RESULTS
    lens, s_exts, perms, in_maps = _prep(query, keys, seq_len, w)

    nc = _nc_cache.get(s_exts)
    if nc is None:
        nc = _build(s_exts)
        _nc_cache[s_exts] = nc

    res = run_bass_kernel_spmd(nc, in_maps, core_ids=list(range(NCORES)))
    LAST_RESULTS = res

    out = np.zeros((B, S), dtype=np.float32)
    for c in range(NCORES):
        dev = np.asarray(res.results[c]["out"]).astype(np.float32)
        pc = perms[c]
        off = 0
        for j in range(NTILES):
            E = s_exts[j]
            rows = pc[j * P : (j + 1) * P]
            out[rows, :E] = dev[:, off : off + E]
            off += E
    # zero masked/padded positions, then fix seq_len==0 rows (uniform).
    out = np.where(np.arange(S)[None, :] < lens[:, None], out, 0.0)
    out[lens == 0, :] = np.float32(1.0 / S)
    return out
